# revision 20
# baseline (speedup 1.0000x reference)
"""DDSP generator Bass kernel for Trainium2, 8-core data parallel.

Sharding: batch 16 -> 8 cores x 2 examples each. Weights replicated.
Per core:
  stage1: main conv stack (fp32 PE) -> h; osc head -> l (amp^2), f (Hz/SR)
  osc bank: 65 resize units per example, re-grouped so 4 units pack one
      PSUM bank via tile_position AND land in a single SBUF accumulator
      (hbig, fp16) whose layout makes the h_out store one DMA per
      (example, row): unit j (1..64) lives at partition 32*((j-1)//16),
      column 384*((j-1)%16).  Unit 0 (head edge) is handled separately.
      Per unit: lerp (Pool/DVE/Act) -> custom DVE op (clip + cumsum +
      wrap) -> Act Sin -> fp16 -> m=2 PE reduce matmul.
  noise branch: 4x (2x-upsample conv k7) via even/odd stride trick with
      fp8e4 DoubleRow matmuls (activations x16, weights x1024 to avoid
      fp8 subnormals; rescaled exactly in the PSUM->SBUF Prelu).  Last
      layer output fp16 -> head conv + Square -> n_l.
  noise FFT: rfft/irfft as matmuls, filter on DVE, overlap-add.
  Emission interleaves noise-conv groups with osc groups so the PE queue
  stays dense while Act/DVE/Pool crunch the oscillator bank.
Host: recombine the two reduce rows with the lerp-weight pattern, pad,
      add noise, normalize, crop (O(output) numpy work only).
"""

import numpy as np
from contextlib import ExitStack

import concourse.bass as bass
import concourse.tile as tile
from concourse import bacc, mybir
from concourse import bass_utils
from concourse import dve_ops
from concourse.dve_spec import Spec, Src0, Src1, C0, C1, C2, scan, minn, maxx, AluOp, lower
from concourse.dve_uop import DveOpSpec

F32 = mybir.dt.float32
F16 = mybir.dt.float16
F8 = mybir.dt.float8e4
F32R = mybir.dt.float32r
AF = mybir.ActivationFunctionType
ALU = mybir.AluOpType
PM = mybir.MatmulPerfMode

SR = 11025.0
UP_LEN = 24576
TOTAL = 16384
WIN = 32
FRAMES = 1024
CROP = 4096
B = 16
NCORES = 8
BPC = 2
T0 = 64
SEG = 384
NGRP = 16           # interior groups per example (units 1..64)
NUNITS = 65
EDGE = 192
HOPAD = UP_LEN + EDGE   # h_out row width (192 tail garbage ignored by host)
LO_U = 20.0 / SR
HI_U = 0.5
MAGIC = 12582912.0
SW = 1024.0         # fp8 weight scale
SA = 16.0           # fp8 activation scale

_CENTERS = np.geomspace(20.0, SR / 2.0 - 20.0, 128).astype(np.float32)
_ERBS = (_CENTERS * np.float32(0.108) + np.float32(24.7)).astype(np.float32)


def _osc_ref(in0, in1, s0, s1, imm2):
    v = np.minimum(np.maximum(in0, np.float32(s0)), np.float32(s1)).astype(np.float32)
    u = np.cumsum(v.astype(np.float64), axis=-1).astype(np.float32)
    y = (u + in1).astype(np.float32)
    r = ((y + np.float32(imm2)) - np.float32(imm2)).astype(np.float32)
    return (y - r).astype(np.float32)


def _register_osc_op():
    if hasattr(dve_ops, "CUSTOM_DVE_OPS_BY_NAME") and \
            "OSC_PHASE_ANT" in dve_ops.CUSTOM_DVE_OPS_BY_NAME:
        return dve_ops.CUSTOM_DVE_OPS_BY_NAME["OSC_PHASE_ANT"]
    body_v = minn(maxx(Src0, C0), C1)
    body_u = scan(AluOp.ADD, body_v)
    body_y = body_u + Src1
    body = body_y - ((body_y + C2) - C2)
    spec = Spec(body=body, reference=_osc_ref)
    sha = {}
    for ver in ("v3",):
        s = DveOpSpec(name="OSC_PHASE_ANT", opcode=1, uops=lower(spec, ver=ver),
                      rd1_en=True)
        sha[ver] = s.sha(ver)
    op = dve_ops.DveOp("OSC_PHASE_ANT", spec, subdim=False, uops_sha=sha)
    dve_ops.OPS.append(op)
    dve_ops.CUSTOM_DVE_SPECS[op.name] = op.spec
    dve_ops._SUB_OPCODE_FOR_NAME[op.name] = max(dve_ops._SUB_OPCODE_FOR_NAME.values()) + 1
    if not hasattr(dve_ops, "CUSTOM_DVE_OPS_BY_NAME"):
        dve_ops.CUSTOM_DVE_OPS_BY_NAME = {}
    dve_ops.CUSTOM_DVE_OPS_BY_NAME[op.name] = op
    return op


_BUILD_CACHE = {}


def _build_program():
    if "nc" in _BUILD_CACHE:
        return _BUILD_CACHE["nc"]
    osc_op = _register_osc_op()

    nc = bacc.Bacc("TRN2", target_bir_lowering=False, debug=False, num_devices=1)

    dI = lambda n, s, dt=F32: nc.dram_tensor(n, s, dt, kind="ExternalInput").ap()
    dO = lambda n, s, dt=F32: nc.dram_tensor(n, s, dt, kind="ExternalOutput").ap()

    x3h = dI("x3h", [128, 2, T0, BPC])
    noi = dI("noi", [BPC, WIN, FRAMES])
    wm0h = dI("wm0h", [128, 2, 512])
    wmLh = [dI(f"wm{i}h", [128, 4, 3, 512]) for i in (1, 2, 3)]
    wfqh = dI("wfqh", [128, 4, 256])
    wn8h = [dI(f"wn8_{l}", [128, 2, 2, 2, 4, 512], F8) for l in range(4)]
    wnhh = dI("wnhh", [128, 4, 34], F16)
    cst = dI("cst", [128, 18])        # 0: cesc, 1: cebi, 2..17: scaled biases
    wth = dI("wth", [128, SEG])
    fgh = dI("fgh", [34, 66])         # [:, 0:32] gmat, [0:32, 32:66] fcat
    eyeh = dI("eyeh", [128, 128])

    h_out = dO("h_out", [2 * BPC, 4, NGRP, SEG], F16)
    h0_out = dO("h0_out", [2 * BPC, EDGE], F16)
    n_out = dO("n_out", [16 * BPC, FRAMES])

    with tile.TileContext(nc) as tc, ExitStack() as ctx:
        cpool = ctx.enter_context(tc.tile_pool(name="consts", bufs=1))
        apool = ctx.enter_context(tc.tile_pool(name="acts", bufs=1))
        fpool = ctx.enter_context(tc.tile_pool(name="fft", bufs=1))
        opool = ctx.enter_context(tc.tile_pool(name="osc", bufs=4))
        hpool = ctx.enter_context(tc.tile_pool(name="hm", bufs=1))
        w1pool = ctx.enter_context(tc.tile_pool(name="w1", bufs=2))
        w2pool = ctx.enter_context(tc.tile_pool(name="w2", bufs=2))
        ps_mm = ctx.enter_context(tc.tile_pool(name="psmm", bufs=3, space="PSUM"))
        ps_osc = ctx.enter_context(tc.tile_pool(name="psosc", bufs=2, space="PSUM"))
        ps_fft = ctx.enter_context(tc.tile_pool(name="psfft", bufs=2, space="PSUM"))

        wt_t = cpool.tile([128, SEG], F32)
        nc.sync.dma_start(wt_t[:], wth[:])
        cst_t = cpool.tile([128, 18], F32)
        nc.sync.dma_start(cst_t[:], cst[:])
        fg_t = cpool.tile([34, 66], F32R)
        nc.sync.dma_start(fg_t[:], fgh[:].bitcast(F32R))
        gmat_t = fg_t[:, 0:32]
        fcat_t = fg_t[0:32, 32:66]
        cesc_t = cst_t[:, 0:1]
        cebi_t = cst_t[:, 1:2]
        eye_t = cpool.tile([128, 128], F32)
        nc.sync.dma_start(eye_t[:], eyeh[:])

        # ================= stage 1 =================
        x_t = apool.tile([128, 2, T0 * BPC], F32, tag="x")
        nc.sync.dma_start(x_t[:], x3h[:].rearrange("c k t b -> c k (t b)"))
        wm0_t = w1pool.tile([128, 2, 512], F32, tag="wm0", bufs=1)
        nc.sync.dma_start(wm0_t[:], wm0h[:])

        NCOL = BPC * T0

        def conv_flip(pm, prelu_dst_list, hbuf_tag, PAD):
            """pm: [128(t b), 512co] accumulated PSUM -> prelu -> SBUF -> 4 PE
            transposes -> psum -> copies into per-m interleaved h tiles."""
            hb = apool.tile([128, 512], F32, tag=hbuf_tag, name=hbuf_tag)
            nc.scalar.activation(hb[:], pm[:], AF.Prelu, bias=0.0, scale=1.0,
                                 alpha=0.2)
            P2 = 2 * PAD
            WID2 = 128 + 2 * P2
            out = []
            for m in range(4):
                pt = ps_mm.tile([128, 128], F32, tag="ptr", name="ptr", bufs=1)
                nc.tensor.matmul(pt[:], hb[:, 128 * m:128 * (m + 1)], eye_t[:],
                                 start=True, stop=True, is_transpose=True)
                ht = prelu_dst_list[m]
                nc.gpsimd.memset(ht[:, 0:P2], 0.0)
                nc.gpsimd.memset(ht[:, P2 + 128:WID2], 0.0)
                if m % 2 == 0:
                    nc.vector.tensor_copy(ht[:, P2:P2 + 128], pt[:])
                else:
                    nc.scalar.copy(ht[:, P2:P2 + 128], pt[:])
                out.append(ht)
            return out

        pm = ps_mm.tile([128, 512], F32, tag="pconv")
        for k in range(2):
            nc.tensor.matmul(pm[:], x_t[:, k, :], wm0_t[:, k, :],
                             start=(k == 0), stop=(k == 1))
        h1 = [apool.tile([128, 132], F32, tag=f"hA{m}", name=f"hA{m}")
              for m in range(4)]
        conv_flip(pm, h1, "hbuf", 1)

        hcur = h1
        for li in range(3):
            wlh = []
            for half in range(2):
                wl = w1pool.tile([128, 2, 3, 512], F32, tag="wmL", name="wmL")
                nc.sync.dma_start(wl[:], wmLh[li][:, 2 * half:2 * half + 2])
                wlh.append(wl)
            last = li == 2
            PAD = 2 if last else 1
            WID2 = 128 + 4 * PAD
            tagp = "hB" if li % 2 == 0 else "hA"
            pm = ps_mm.tile([128, 512], F32, tag="pconv")
            i_mm = 0
            for k in range(4):
                wl = wlh[k // 2]
                for tap in range(3):
                    nc.tensor.matmul(pm[:], hcur[k][:, 2 * tap:2 * tap + 128],
                                     wl[:, k % 2, tap, :],
                                     start=(i_mm == 0), stop=(i_mm == 11))
                    i_mm += 1
            hnxt = [apool.tile([128, WID2], F32,
                               tag=(f"h4_{m}" if last else f"{tagp}{m}"),
                               name=f"h_{li}_{m}")
                    for m in range(4)]
            conv_flip(pm, hnxt, "hbuf", PAD)
            hcur = hnxt
        h4 = hcur   # [128, 136] interleaved (t b), pad 2 frames each side

        wfq_t = w1pool.tile([128, 4, 256], F32, tag="wfq", bufs=1)
        nc.sync.dma_start(wfq_t[:], wfqh[:])
        l_sb = apool.tile([128, 128], F32, tag="l_sb")
        f_sb = apool.tile([128, 128], F32, tag="f_sb")
        for m in range(2):
            pmf = ps_mm.tile([128, 128], F32, tag="pconv", name="pconv")
            for k in range(4):
                nc.tensor.matmul(pmf[:], wfq_t[:, k, 128 * m:128 * (m + 1)],
                                 h4[k][:, 4:132],
                                 start=(k == 0), stop=(k == 3))
            if m == 0:
                nc.scalar.activation(l_sb[:], pmf[:], AF.Square)
            else:
                tanh_t = apool.tile([128, 128], F32, tag="tanh")
                nc.scalar.activation(tanh_t[:], pmf[:], AF.Tanh)
                nc.scalar.activation(f_sb[:], tanh_t[:],
                                     AF.Identity, bias=cebi_t, scale=cesc_t)

        # ================= osc prep =================
        flo_u, df_u, c_u, l2_u = [], [], [], []
        for ex in range(BPC):
            f_ex = f_sb[:, ex::2]
            l_ex = l_sb[:, ex::2]

            flo = apool.tile([128, NUNITS], F32, tag=f"flo{ex}")
            nc.gpsimd.tensor_copy(flo[:, 0:1], f_ex[:, 0:1])
            nc.gpsimd.tensor_copy(flo[:, 1:65], f_ex[:, 0:64])
            dfu = apool.tile([128, NUNITS], F32, tag=f"dfu{ex}")
            nc.gpsimd.memset(dfu[:, 0:1], 0.0)
            nc.gpsimd.memset(dfu[:, 64:65], 0.0)
            nc.gpsimd.tensor_tensor(dfu[:, 1:64], f_ex[:, 1:64], f_ex[:, 0:63], ALU.subtract)

            l2t = apool.tile([128, NUNITS, 2], F16, tag=f"l2{ex}")
            nc.gpsimd.tensor_copy(l2t[:, 0:1, 0], l_ex[:, 0:1])
            nc.gpsimd.tensor_copy(l2t[:, 1:65, 0], l_ex[:, 0:64])
            nc.gpsimd.memset(l2t[:, 0:1, 1], 0.0)
            nc.gpsimd.memset(l2t[:, 64:65, 1], 0.0)
            nc.gpsimd.tensor_tensor(l2t[:, 1:64, 1], l_ex[:, 1:64], l_ex[:, 0:63], ALU.subtract)

            a = f_ex[:, 0:63]
            b_ = f_ex[:, 1:64]

            def T63(tag):
                return apool.tile([128, 63], F32, tag=tag, name=tag)

            alo = T63("p_alo")
            nc.vector.tensor_tensor(alo[:], a, b_, ALU.min)
            ahi = T63("p_ahi")
            nc.vector.tensor_tensor(ahi[:], a, b_, ALU.max)
            dd = T63("p_dd")
            nc.gpsimd.tensor_tensor(dd[:], ahi[:], alo[:], ALU.subtract)
            ddc = T63("p_ddc")
            nc.vector.tensor_scalar(ddc[:], dd[:], 1e-30, None, ALU.max)
            inv = T63("p_inv")
            nc.vector.reciprocal(inv[:], ddc[:])
            dd768 = T63("p_dd768")
            nc.gpsimd.tensor_scalar(dd768[:], dd[:], float(1.0 / 768.0), None, ALU.mult)

            t1 = T63("p_t1")
            nc.gpsimd.tensor_scalar(t1[:], alo[:], LO_U, -384.0, ALU.subtract, ALU.mult)
            c1 = T63("p_c1")
            nc.gpsimd.tensor_tensor(c1[:], t1[:], inv[:], ALU.mult)
            nc.vector.tensor_scalar(c1[:], c1[:], 0.0, 384.0, ALU.max, ALU.min)
            nc.gpsimd.tensor_scalar(c1[:], c1[:], MAGIC, MAGIC, ALU.add, ALU.subtract)
            lo_alo = T63("p_loalo")
            nc.gpsimd.tensor_scalar(lo_alo[:], alo[:], LO_U, -1.0, ALU.subtract, ALU.mult)
            u1 = T63("p_u1")
            nc.gpsimd.tensor_tensor(u1[:], dd768[:], c1[:], ALU.mult)
            nc.gpsimd.tensor_tensor(u1[:], lo_alo[:], u1[:], ALU.subtract)
            s1c = T63("p_s1c")
            nc.gpsimd.tensor_tensor(s1c[:], c1[:], u1[:], ALU.mult)

            t2 = T63("p_t2")
            nc.gpsimd.tensor_scalar(t2[:], ahi[:], HI_U, 384.0, ALU.subtract, ALU.mult)
            c2 = T63("p_c2")
            nc.gpsimd.tensor_tensor(c2[:], t2[:], inv[:], ALU.mult)
            nc.vector.tensor_scalar(c2[:], c2[:], 0.0, 384.0, ALU.max, ALU.min)
            nc.gpsimd.tensor_scalar(c2[:], c2[:], MAGIC, MAGIC, ALU.add, ALU.subtract)
            ahi_hi = T63("p_ahihi")
            nc.gpsimd.tensor_scalar(ahi_hi[:], ahi[:], HI_U, None, ALU.subtract)
            u2 = T63("p_u2")
            nc.gpsimd.tensor_tensor(u2[:], dd768[:], c2[:], ALU.mult)
            nc.gpsimd.tensor_tensor(u2[:], ahi_hi[:], u2[:], ALU.subtract)
            s2c = T63("p_s2c")
            nc.gpsimd.tensor_tensor(s2c[:], c2[:], u2[:], ALU.mult)

            tall = apool.tile([128, 64], F32, tag="p_tall")
            slin = T63("p_slin")
            nc.gpsimd.tensor_tensor(slin[:], a, b_, ALU.add)
            nc.gpsimd.tensor_scalar(slin[:], slin[:], 192.0, None, ALU.mult)
            nc.gpsimd.tensor_tensor(tall[:, 1:64], slin[:], s1c[:], ALU.add)
            nc.gpsimd.tensor_tensor(tall[:, 1:64], tall[:, 1:64], s2c[:], ALU.subtract)
            nc.vector.tensor_scalar(tall[:, 0:1], f_ex[:, 0:1], LO_U, HI_U, ALU.max, ALU.min)
            nc.gpsimd.tensor_scalar(tall[:, 0:1], tall[:, 0:1], 192.0, None, ALU.mult)
            trnd = apool.tile([128, 64], F32, tag="p_trnd")
            nc.gpsimd.tensor_scalar(trnd[:], tall[:], MAGIC, MAGIC, ALU.add, ALU.subtract)
            nc.gpsimd.tensor_tensor(tall[:], tall[:], trnd[:], ALU.subtract)
            cinc = apool.tile([128, 64], F32, tag="p_cinc")
            nc.vector.tensor_tensor_scan(cinc[:], tall[:], tall[:], 0.0, ALU.add, ALU.bypass)
            cu = apool.tile([128, NUNITS], F32, tag=f"cu{ex}")
            nc.gpsimd.memset(cu[:, 0:1], 0.0)
            nc.vector.tensor_copy(cu[:, 1:65], cinc[:])

            flo_u.append(flo)
            df_u.append(dfu)
            c_u.append(cu)
            l2_u.append(l2t)

        # ================= h4 -> fp8 (scaled) for noise branch =================
        h48 = []
        for P in range(2):
            t = apool.tile([128, 2, BPC, 68], F8, tag=f"h48_{P}")
            for s in range(2):
                for ex in range(BPC):
                    nc.gpsimd.tensor_scalar(t[:, s, ex, :],
                                              h4[2 * P + s][:, ex::2],
                                              SA, None, ALU.mult)
            h48.append(t)

        # early FFT input prep
        nz2_t = []
        for ex in range(BPC):
            nzt = fpool.tile([WIN, FRAMES], F32, tag=f"nz{ex}", name="nzt")
            nc.sync.dma_start(nzt[:], noi[ex, :, :])
            nz2 = fpool.tile([WIN, FRAMES], F32R, tag=f"nz2{ex}", name="nz2")
            nc.gpsimd.tensor_scalar(nz2[:], nzt[:], 2.0, -1.0, ALU.mult, ALU.add)
            nz2_t.append(nz2)

        # ================= task thunks =================
        two_pi = float(2.0 * np.pi)

        def units_of(ex, g):
            if g < 0:
                return [(0, EDGE)]
            return [(1 + g + NGRP * gi, EDGE if (1 + g + NGRP * gi) == 64 else SEG)
                    for gi in range(4)]

        tiles_fu, tiles_ph, tiles_s16, tiles_pm = {}, {}, {}, {}
        done_count = [0] * BPC

        def st_lerp(ex, g):
            fus = []
            for gi, (j, wdt) in enumerate(units_of(ex, g)):
                fu = opool.tile([128, SEG], F32, tag="fu", bufs=8, name="fu")
                dfa = df_u[ex][:, j:j + 1]
                floa = flo_u[ex][:, j:j + 1]
                eng = nc.vector if gi == 0 else nc.gpsimd
                eng.tensor_scalar(fu[:, 0:wdt], wt_t[:, 0:wdt], dfa, floa,
                                  ALU.mult, ALU.add)
                fus.append(fu)
            tiles_fu[(ex, g)] = fus

        def st_scan(ex, g):
            fus = tiles_fu.pop((ex, g))
            ph = opool.tile([128, 4 * SEG], F32, tag="ph4", bufs=2, name="ph4")
            for gi, (j, wdt) in enumerate(units_of(ex, g)):
                nc.vector._custom_dve(
                    osc_op, out=ph[:, SEG * gi:SEG * gi + wdt], in0=fus[gi][:, 0:wdt],
                    in1=c_u[ex][:, j:j + 1].to_broadcast((128, wdt)),
                    s0=LO_U, s1=HI_U, imm2=MAGIC)
                if g >= 0 and wdt < SEG:
                    nc.gpsimd.memset(ph[:, SEG * gi + wdt:SEG * (gi + 1)], 0.0)
            tiles_ph[(ex, g)] = ph

        def st_sin(ex, g):
            ph = tiles_ph.pop((ex, g))
            s16 = opool.tile([128, 4 * SEG], F16, tag="s16", bufs=2, name="s16")
            W = 4 * SEG if g >= 0 else EDGE
            nc.scalar.activation(s16[:, 0:W], ph[:, 0:W], AF.Sin, bias=0.0, scale=two_pi)
            tiles_s16[(ex, g)] = s16

        def st_mm(ex, g):
            s16 = tiles_s16.pop((ex, g))
            pm4 = ps_osc.tile([128, SEG], F32, tag="pm4", name="pm4")
            for gi, (j, wdt) in enumerate(units_of(ex, g)):
                nc.tensor.matmul(pm4[32 * gi:32 * gi + 2, 0:wdt],
                                 l2_u[ex][:, j, :],
                                 s16[:, SEG * gi:SEG * gi + wdt],
                                 start=True, stop=True,
                                 tile_position=(0, 32 * gi))
            tiles_pm[(ex, g)] = pm4

        def st_copy(ex, g):
            pm4 = tiles_pm.pop((ex, g))
            hs = hpool.tile([128, SEG], F16, tag="hs", bufs=3, name="hs")
            if g >= 0:
                if g % 2 == 0:
                    nc.vector.tensor_copy(hs[:], pm4[:])
                else:
                    nc.scalar.copy(hs[:], pm4[:])
                for r in range(2):
                    nc.sync.dma_start(h_out[2 * ex + r, :, g, :], hs[r::32, :])
            else:
                nc.vector.tensor_copy(hs[0:2, 0:EDGE], pm4[0:2, 0:EDGE])
                nc.sync.dma_start(h0_out[2 * ex:2 * ex + 2, :], hs[0:2, 0:EDGE])

        # noise branch thunks
        state = {"ycur": h48, "TI": T0}

        def noise_wdma(li):
            wgt = []
            for eo in range(2):
                w = w2pool.tile([128, 2, 2, 4, 512], F8, tag="wn8", name="wn8")
                nc.sync.dma_start(w[:], wn8h[li][:, eo])
                wgt.append(w)
            state["wgt"] = wgt
            TI = state["TI"]
            TOUT = TI * 2
            WIDO = TOUT + 4
            last = li == 3
            if not last:
                ynxt = [apool.tile([128, 2, BPC, WIDO], F8, tag=f"yl{li}_{P}", name=f"yl{li}_{P}")
                        for P in range(2)]
                for P in range(2):
                    nc.gpsimd.memset(ynxt[P][:, :, :, 0:2], 0.0)
                    nc.gpsimd.memset(ynxt[P][:, :, :, WIDO - 2:WIDO], 0.0)
            else:
                ynxt = [apool.tile([128, BPC, WIDO], F16, tag=f"y4_{m}", name=f"y4_{m}")
                        for m in range(4)]
                for m in range(4):
                    nc.gpsimd.memset(ynxt[m][:, :, 0:2], 0.0)
                    nc.gpsimd.memset(ynxt[m][:, :, WIDO - 2:WIDO], 0.0)
            state["ynxt"] = ynxt

        def noise_group(li, eo, m, ex):
            TI = state["TI"]
            wgt = state["wgt"]
            ynxt = state["ynxt"]
            ycur = state["ycur"]
            last = li == 3
            pm = ps_mm.tile([128, 512], F32, tag="pconv")
            i_mm = 0
            for P in range(2):
                for tap in range(4):
                    off = tap + eo
                    nc.tensor.matmul(pm[:, 0:TI],
                                     wgt[eo][:, P, :, tap, 128 * m:128 * (m + 1)],
                                     ycur[P][:, :, ex, off:off + TI],
                                     start=(i_mm == 0), stop=(i_mm == 7),
                                     perf_mode=PM.DoubleRow)
                    i_mm += 1
            bias_ap = cst_t[:, 2 + 4 * li + m:2 + 4 * li + m + 1]
            if not last:
                dst = ynxt[m // 2][:, m % 2, ex, 2 + eo:2 + eo + 2 * TI:2]
                scl = 1.0 / SW
            else:
                dst = ynxt[m][:, ex, 2 + eo:2 + eo + 2 * TI:2]
                scl = 1.0 / (SW * SA)
            nc.scalar.activation(dst, pm[:, 0:TI], AF.Prelu,
                                 bias=bias_ap, scale=scl, alpha=0.2)

        def noise_adv():
            state["ycur"] = state["ynxt"]
            state["TI"] = state["TI"] * 2

        noise_tasks = []
        for li in range(4):
            noise_tasks.append(lambda li=li: noise_wdma(li))
            if li < 3:
                for eo in range(2):
                    for m in range(4):
                        for ex in range(BPC):
                            noise_tasks.append(
                                lambda li=li, eo=eo, m=m, ex=ex: noise_group(li, eo, m, ex))
            else:
                for ex in range(BPC):
                    for eo in range(2):
                        for m in range(4):
                            noise_tasks.append(
                                lambda li=li, eo=eo, m=m, ex=ex: noise_group(li, eo, m, ex))
            noise_tasks.append(noise_adv)

        # head + fft thunks (after noise)
        def head_fft():
            ycur = state["ycur"]   # y4 fp16 tiles [128, BPC, 1028]
            wh_t = w2pool.tile([128, 4, 34], F16, tag="wnh", bufs=1)
            nc.sync.dma_start(wh_t[:], wnhh[:])
            nl_sb = []
            for ex in range(BPC):
                nlt = apool.tile([34, FRAMES], F32, tag=f"nl{ex}")
                for half in range(2):
                    pm = ps_fft.tile([34, 512], F32, tag="pfft")
                    for k in range(4):
                        nc.tensor.matmul(pm[:],
                                         wh_t[:, k, :],
                                         ycur[k][:, ex, 2 + 512 * half:2 + 512 * (half + 1)],
                                         start=(k == 0), stop=(k == 3))
                    nc.scalar.activation(nlt[:, 512 * half:512 * (half + 1)], pm[:],
                                         AF.Square)
                nl_sb.append(nlt)

            for ex in range(BPC):
                fcs = fpool.tile([34, FRAMES], F32R, tag="fcs")
                for half in range(2):
                    pm = ps_fft.tile([34, 512], F32, tag="pfft")
                    nc.tensor.matmul(pm[:], fcat_t,
                                     nz2_t[ex][:, 512 * half:512 * (half + 1)],
                                     start=True, stop=True)
                    nc.vector.tensor_tensor(fcs[:, 512 * half:512 * (half + 1)], pm[:],
                                            nl_sb[ex][:, 512 * half:512 * (half + 1)],
                                            ALU.mult)
                frsA = fpool.tile([16, FRAMES], F32, tag="frsA")
                frsB = fpool.tile([16, FRAMES], F32, tag="frsB")
                for half in range(2):
                    pm = ps_fft.tile([34, 512], F32, tag="pfft")
                    nc.tensor.matmul(pm[0:16, :], gmat_t[:, 0:16],
                                     fcs[:, 512 * half:512 * (half + 1)],
                                     start=True, stop=True)
                    nc.scalar.copy(frsA[:, 512 * half:512 * (half + 1)], pm[0:16, :])
                    pm2 = ps_fft.tile([34, 512], F32, tag="pfft")
                    nc.tensor.matmul(pm2[0:16, :], gmat_t[:, 16:32],
                                     fcs[:, 512 * half:512 * (half + 1)],
                                     start=True, stop=True)
                    nc.vector.tensor_copy(frsB[:, 512 * half:512 * (half + 1)], pm2[0:16, :])
                nsb = fpool.tile([16, FRAMES], F32, tag="nsb")
                nc.vector.tensor_copy(nsb[:, 0:1], frsA[:, 0:1])
                nc.vector.tensor_tensor(nsb[:, 1:FRAMES], frsA[:, 1:FRAMES],
                                        frsB[:, 0:FRAMES - 1], ALU.add)
                nc.sync.dma_start(n_out[16 * ex:16 * (ex + 1), :], nsb[:])

        noise_tasks.append(head_fft)

        # ================= interleaved emission (software pipeline) =======
        G = [(ex, g) for ex in range(BPC) for g in list(range(NGRP)) + [-1]]
        NW = len(G) + 4
        ni = 0
        for s in range(NW):
            if s < len(G):
                st_lerp(*G[s])
            if 1 <= s <= len(G):
                st_scan(*G[s - 1])
            if 2 <= s <= len(G) + 1:
                st_sin(*G[s - 2])
            if 3 <= s <= len(G) + 2:
                st_mm(*G[s - 3])
            if 4 <= s <= len(G) + 3:
                st_copy(*G[s - 4])
            target = (s + 1) * len(noise_tasks) // NW
            while ni < min(target, len(noise_tasks)):
                noise_tasks[ni]()
                ni += 1
        assert ni == len(noise_tasks)

    nc.compile()
    _BUILD_CACHE["nc"] = nc
    return nc


_W_PAT = None


def _wpat():
    global _W_PAT
    if _W_PAT is None:
        w = np.zeros(UP_LEN, np.float32)
        kk = ((np.arange(SEG) + 0.5) / SEG).astype(np.float32)
        for s in range(63):
            w[EDGE + SEG * s: EDGE + SEG * (s + 1)] = kk
        _W_PAT = w
    return _W_PAT


def _prep_shared(inputs):
    E4 = mybir.dt.np(F8)
    d = {}
    # stage-1 weights, one DMA-ready array each
    wm0 = np.asarray(inputs["w_main0"], np.float32)[:, :, 0]      # [co512, ci256]
    d["wm0h"] = np.ascontiguousarray(wm0.T.reshape(2, 128, 512).transpose(1, 0, 2))
    for i in (1, 2, 3):
        W = np.asarray(inputs[f"w_main{i}"], np.float32)          # [co, ci, 3]
        a2 = W.transpose(1, 2, 0).reshape(4, 128, 3, 512)         # [k, p, a, o]
        d[f"wm{i}h"] = np.ascontiguousarray(a2.transpose(1, 0, 2, 3))
    wfq = np.asarray(inputs["w_freq"], np.float32)[:, :, 0]       # [256co, 512ci]
    d["wfqh"] = np.ascontiguousarray(wfq.T.reshape(4, 128, 256).transpose(1, 0, 2))
    # noise conv weights: fp8, [p, eo, P, s, tap, co], scaled by SW
    for l in range(4):
        W = np.asarray(inputs[f"w_nl{l}"], np.float32)
        We = np.stack([W[:, :, 0], W[:, :, 1] + W[:, :, 2],
                       W[:, :, 3] + W[:, :, 4], W[:, :, 5] + W[:, :, 6]], -1)
        Wo = np.stack([W[:, :, 0] + W[:, :, 1], W[:, :, 2] + W[:, :, 3],
                       W[:, :, 4] + W[:, :, 5], W[:, :, 6]], -1)
        arr = np.stack([We, Wo], 0)                     # [eo, co, ci, tap]
        a2 = arr.transpose(2, 0, 3, 1)                  # [ci, eo, tap, co]
        a2 = a2.reshape(2, 2, 128, 2, 4, 512)           # [P, s, p, eo, tap, co]
        a2 = a2.transpose(2, 3, 0, 1, 4, 5)             # [p, eo, P, s, tap, co]
        d[f"wn8_{l}"] = np.ascontiguousarray((a2 * np.float32(SW))).astype(E4)
    wh = np.asarray(inputs["w_noise_loud"], np.float32)[:, :, 0].T   # [512, 17]
    whd = np.concatenate([wh, wh], 1).astype(np.float16)             # [512, 34]
    d["wnhh"] = np.ascontiguousarray(whd.reshape(4, 128, 34).transpose(1, 0, 2))
    cstv = np.zeros((128, 18), np.float32)
    cstv[:, 0] = (0.5 * _ERBS / SR).astype(np.float32)
    cstv[:, 1] = (_CENTERS / SR).astype(np.float32)
    for l in range(4):
        bl = np.asarray(inputs[f"b_nl{l}"], np.float32).reshape(4, 128)
        sc = SA if l < 3 else 1.0
        for m in range(4):
            cstv[:, 2 + 4 * l + m] = bl[m] * sc
    d["cst"] = cstv
    d["wth"] = np.ascontiguousarray(
        np.broadcast_to(((np.arange(SEG) + 0.5) / SEG).astype(np.float32), (128, SEG)))
    k = np.arange(WIN)[:, None].astype(np.float64)
    j = np.arange(17)[None, :].astype(np.float64)
    fre = np.cos(-2 * np.pi * k * j / WIN) / np.sqrt(WIN)
    fim = np.sin(-2 * np.pi * k * j / WIN) / np.sqrt(WIN)
    fcat = np.concatenate([fre, fim], 1).astype(np.float32)          # [32, 34]
    t = np.arange(WIN)[None, :].astype(np.float64)
    jj = np.arange(17)[:, None].astype(np.float64)
    wgt = np.where((jj == 0) | (jj == 16), 1.0, 2.0)
    gre = wgt * np.cos(2 * np.pi * jj * t / WIN) / np.sqrt(WIN)
    gim = -wgt * np.sin(2 * np.pi * jj * t / WIN) / np.sqrt(WIN)
    gmat = np.concatenate([gre, gim], 0).astype(np.float32)          # [34, 32]
    fg = np.zeros((34, 66), np.float32)
    fg[:, 0:32] = gmat
    fg[0:32, 32:66] = fcat
    d["fgh"] = fg
    d["eyeh"] = np.eye(128, dtype=np.float32)
    return d


def _in_maps(inputs):
    shared = _prep_shared(inputs)
    x = np.asarray(inputs["x"], np.float32)
    noise = np.asarray(inputs["noise"], np.float32)
    maps = []
    for c in range(NCORES):
        m = dict(shared)
        xc = x[BPC * c:BPC * (c + 1)]                    # [2, 256, 64]
        m["x3h"] = np.ascontiguousarray(
            xc.reshape(BPC, 2, 128, T0).transpose(2, 1, 3, 0))
        m["noi"] = np.ascontiguousarray(
            noise[BPC * c:BPC * (c + 1)].transpose(0, 2, 1))
        maps.append(m)
    return maps


def _assemble(results):
    wpat = _wpat()
    out = np.empty((B, 1, TOTAL), np.float32)
    for c in range(NCORES):
        h_o = results[c]["h_out"].astype(np.float32)
        h0 = results[c]["h0_out"].astype(np.float32)
        n_o = results[c]["n_out"]
        for ex in range(BPC):
            bidx = BPC * c + ex
            lo = np.empty(UP_LEN, np.float32)
            dl = np.empty(UP_LEN, np.float32)
            lo[:EDGE] = h0[2 * ex]
            dl[:EDGE] = h0[2 * ex + 1]
            lo[EDGE:] = h_o[2 * ex].reshape(-1)[:UP_LEN - EDGE]
            dl[EDGE:] = h_o[2 * ex + 1].reshape(-1)[:UP_LEN - EDGE]
            sig = lo + wpat * dl
            nzf = np.ascontiguousarray(n_o[16 * ex:16 * (ex + 1)].T).reshape(TOTAL)
            sig[CROP:CROP + TOTAL] += nzf
            mx = np.abs(sig).max()
            out[bidx, 0] = sig[CROP:CROP + TOTAL] / (mx + np.float32(1e-8))
    return out


def kernel(**inputs) -> np.ndarray:
    nc = _build_program()
    maps = _in_maps(inputs)
    res = bass_utils.run_bass_kernel_spmd(nc, maps, core_ids=list(range(NCORES)))
    return _assemble([res.results[c] for c in range(NCORES)])
